# revision 1
# baseline (speedup 1.0000x reference)
"""Trainium2 Bass kernel for the DecoderRNN-DTP problem.

Math (per reference):
  x   = relu(dtp_features @ W_enc.T + b_enc)              [B, H]
  gi_l = x @ W_ih_l.T + b_ih_l                            [B, 3H]   (constant over steps)
  60 steps, each step threads one hidden state h through 3 GRU cells:
      gh = h @ W_hh_l.T + b_hh_l
      r = sig(gi_r + gh_r); z = sig(gi_z + gh_z)
      n = tanh(gi_n + r * gh_n);  h = (1-z)*n + z*h
  y_t = h @ W_out.T + b_out                               [B, 4]
  out[b, o*60+t] = y_t[b, o]

Distribution: data-parallel over batch, 8 cores x 256 rows; weights replicated.

Layout is fully transposed on device: hidden/gate dims on SBUF partitions,
batch is the free/moving dim.  The 256-row per-core batch is further split
into two 128-row streams so the tensor engine can run one stream's W_hh
matmuls while the vector/scalar/gpsimd engines chew the other stream's gate
math -- the GRU recurrence is strictly serial within a stream, so two
independent streams are what buys engine overlap.

Precision: the recurrent W_hh matmuls use bf16 weights x bf16 h-copy
(numpy study: rel err ~5e-4 end-to-end because gate math and the h state
stay fp32); everything else (encoder, gi, preloads, y head, gate math) is
fp32/fp32r (fp22 multiply).  Gate constants (gi + biases) are preloaded
into PSUM by bank-wide identity matmuls so the W_hh accumulation lands on
them; exactly one start=True per PSUM bank (start clears has_written for
the whole bank).
"""

import numpy as np
import ml_dtypes

import concourse.bass as bass
import concourse.bacc as bacc
import concourse.tile as tile
import concourse.mybir as mybir
from concourse.bass_utils import run_bass_kernel_spmd

H = 512
B = 2048
STEPS = 60
NCORES = 8
BL = B // NCORES   # 256 batch rows per core
HB = BL // 2       # 128 rows per stream
KT = H // 128      # 4 contraction tiles over hidden dim
GT = 3 * H // 128  # 12 gate tiles
ET = 2048 // 128   # 16 contraction tiles over encoder input dim
YCHUNK = 6         # steps buffered between output DMAs

F32 = mybir.dt.float32
F32R = mybir.dt.float32r
BF16 = mybir.dt.bfloat16
AF = mybir.ActivationFunctionType
OP = mybir.AluOpType

_BUILT = None
LAST_RESULTS = None
LAST_IN_MAPS = None
REPS = 1  # timing builds repeat the step loop to cancel dispatch overhead


def r32(ap):
    return ap.bitcast(F32R)


def flat(ap):
    return ap.rearrange("p a b -> p (a b)")


def _build(reps=None):
    reps = REPS if reps is None else reps
    nc = bacc.Bacc("TRN2", target_bir_lowering=False, debug=False,
                   num_devices=NCORES)

    # ---- DRAM parameters (pre-laid-out on host) ----
    dtpT_d = nc.dram_tensor("dtpT", [ET, 128, BL], F32R, kind="ExternalInput")
    wencT_d = nc.dram_tensor("wencT", [ET, 128, H], F32R, kind="ExternalInput")
    wihT_d = nc.dram_tensor("wihT", [3, KT, 128, 3 * H], F32R, kind="ExternalInput")
    whhT_d = nc.dram_tensor("whhT", [3, KT, 128, 3 * H], BF16, kind="ExternalInput")
    woutT_d = nc.dram_tensor("woutT", [KT, 128, 4], F32R, kind="ExternalInput")
    # biases packed with layer l at partition 32*l (K=1 matmul base_partition
    # must be 0/32/64); bmi: p0 = b_enc, p32 = b_out
    brz_d = nc.dram_tensor("brz", [128, 2 * H], F32R, kind="ExternalInput")
    bni_d = nc.dram_tensor("bni", [128, H], F32R, kind="ExternalInput")
    bmi_d = nc.dram_tensor("bmi", [128, H], F32R, kind="ExternalInput")
    ident_d = nc.dram_tensor("ident", [128, 128], F32R, kind="ExternalInput")
    ones_d = nc.dram_tensor("ones", [128, BL], F32R, kind="ExternalInput")
    # b_hh n-chunk broadcast across a 128-wide half-batch, per layer
    bnhbc_d = nc.dram_tensor("bnhbc", [3, 128, 4, HB], F32R, kind="ExternalInput")
    h0_d = nc.dram_tensor("h0", [128, KT, BL], F32R, kind="ExternalInput")
    h0b_d = nc.dram_tensor("h0b", [128, KT, BL], BF16, kind="ExternalInput")
    y_d = nc.dram_tensor("y", [STEPS, 4, BL], F32, kind="ExternalOutput")

    with tile.TileContext(nc) as tc:
        with (
            tc.tile_pool(name="consts", bufs=1) as consts,
            tc.tile_pool(name="whh", bufs=1) as whhp,
            tc.tile_pool(name="wstream", bufs=3) as wstream,
            tc.tile_pool(name="wihstream", bufs=2) as wihstream,
            tc.tile_pool(name="gpool", bufs=3) as gpool,
            tc.tile_pool(name="hpool", bufs=2) as hpool,
            tc.tile_pool(name="work", bufs=1) as work,
            tc.tile_pool(name="ybuf", bufs=2) as ybufp,
            tc.tile_pool(name="pg", bufs=1, space=bass.MemorySpace.PSUM) as pg,
            tc.tile_pool(name="py", bufs=2, space=bass.MemorySpace.PSUM) as pyp,
        ):
            # ---- constants ----
            ident = consts.tile([128, 128], F32R)
            nc.sync.dma_start(ident[:, :], ident_d[:, :])
            ones = consts.tile([128, BL], F32R)
            nc.sync.dma_start(ones[:, :], ones_d[:, :])
            brz = consts.tile([128, 2 * H], F32R)
            nc.sync.dma_start(brz[:, :], brz_d[:, :])
            bni = consts.tile([128, H], F32R)
            nc.sync.dma_start(bni[:, :], bni_d[:, :])
            bmi = consts.tile([128, H], F32R)
            nc.sync.dma_start(bmi[:, :], bmi_d[:, :])
            bnhbc = consts.tile([128, 3, 4, HB], F32R)
            for l in range(3):
                nc.sync.dma_start(bnhbc[:, l, :, :], bnhbc_d[l, :, :, :])
            woutT = consts.tile([128, KT, 4], F32R)
            for kt in range(KT):
                nc.sync.dma_start(woutT[:, kt, :], woutT_d[kt, :, :])
            zeros2 = consts.tile([128, KT, HB], F32R)
            nc.sync.dma_start(zeros2[:, :, :], h0_d[:, :, 0:HB])

            # ---- recurrent weights, bf16 (resident) ----
            whhT = whhp.tile([128, 3, KT, 3 * H], BF16)
            for l in range(3):
                for kt in range(KT):
                    nc.sync.dma_start(whhT[:, l, kt, :], whhT_d[l, kt, :, :])

            # ---- encoder: xT = relu(W_enc @ dtp.T + b_enc), per stream ----
            psx = [pg.tile([128, KT, HB], F32, tag=f"g{s}", name=f"psx{s}")
                   for s in (0, 1)]
            for s in (0, 1):
                nc.tensor.matmul(flat(psx[s][:, :, :]), r32(ident[:, :]),
                                 r32(flat(zeros2[:, :, :])),
                                 start=True, stop=False)
            for et in range(ET):
                dtc = wstream.tile([128, BL], F32R, tag="dtp")
                nc.sync.dma_start(dtc[:, :], dtpT_d[et, :, :])
                wec = wstream.tile([128, H], F32R, tag="wenc")
                nc.sync.dma_start(wec[:, :], wencT_d[et, :, :])
                for s in (0, 1):
                    for ht in range(KT):
                        nc.tensor.matmul(psx[s][:, ht, :],
                                         r32(wec[:, ht * 128:(ht + 1) * 128]),
                                         r32(dtc[:, s * HB:(s + 1) * HB]),
                                         start=False, stop=False)
            for s in (0, 1):
                for ht in range(KT):
                    nc.tensor.matmul(psx[s][:, ht, :],
                                     r32(bmi[0:1, ht * 128:(ht + 1) * 128]),
                                     r32(ones[0:1, 0:HB]),
                                     start=False, stop=True)
            xT = work.tile([128, KT, BL], F32R, tag="xT")
            for s in (0, 1):
                nc.scalar.activation(xT[:, :, s * HB:(s + 1) * HB],
                                     psx[s][:, :, :], AF.Relu)

            # ---- G_l = gi_l (+ rz: +b_ih+b_hh ; n: +b_ih), per stream ----
            G = []
            for l in range(3):
                psg = [pg.tile([128, GT, HB], F32, tag=f"g{s}",
                                name=f"psg{l}_{s}") for s in (0, 1)]
                for s in (0, 1):
                    for bank in range(3):
                        nc.tensor.matmul(
                            flat(psg[s][:, 4 * bank:4 * bank + 4, :]),
                            r32(ident[:, :]), r32(flat(zeros2[:, :, :])),
                            start=True, stop=False)
                for kt in range(KT):
                    wic = wihstream.tile([128, 3 * H], F32R, tag="wih")
                    nc.sync.dma_start(wic[:, :], wihT_d[l, kt, :, :])
                    for s in (0, 1):
                        for gt in range(GT):
                            nc.tensor.matmul(
                                psg[s][:, gt, :],
                                r32(wic[:, gt * 128:(gt + 1) * 128]),
                                r32(xT[:, kt, s * HB:(s + 1) * HB]),
                                start=False, stop=False)
                for s in (0, 1):
                    for gt in range(8):
                        nc.tensor.matmul(psg[s][:, gt, :],
                                         r32(brz[32 * l:32 * l + 1,
                                                 gt * 128:(gt + 1) * 128]),
                                         r32(ones[32 * l:32 * l + 1, 0:HB]),
                                         start=False, stop=True)
                    for j in range(4):
                        nc.tensor.matmul(psg[s][:, 8 + j, :],
                                         r32(bni[32 * l:32 * l + 1,
                                                 j * 128:(j + 1) * 128]),
                                         r32(ones[32 * l:32 * l + 1, 0:HB]),
                                         start=False, stop=True)
                g = gpool.tile([128, 2, GT, HB], F32R, tag="G")
                for s in (0, 1):
                    nc.scalar.copy(g[:, s, :, :], psg[s][:, :, :])
                G.append(g)

            # ---- recurrent loop: 60 steps x 3 GRU cells, 2 streams ----
            h32 = hpool.tile([128, KT, BL], F32R, tag="h32")
            nc.sync.dma_start(h32[:, :, :], h0_d[:, :, :])
            hb = hpool.tile([128, KT, BL], BF16, tag="hb")
            nc.sync.dma_start(hb[:, :, :], h0b_d[:, :, :])

            ybuf = None
            for t in range(STEPS * reps):
                t = t % STEPS
                for l in range(3):
                    h32n = hpool.tile([128, KT, BL], F32R, tag="h32")
                    hbn = hpool.tile([128, KT, BL], BF16, tag="hb")
                    for s in (0, 1):
                        c0, c1 = s * HB, (s + 1) * HB
                        ps = pg.tile([128, GT, HB], F32, tag=f"g{s}")
                        # bank-wide gate-constant preloads (one start=True per
                        # 2KB PSUM bank -- start clears the whole bank's
                        # has_written bits)
                        for bank in range(2):
                            nc.tensor.matmul(
                                flat(ps[:, 4 * bank:4 * bank + 4, :]),
                                r32(ident[:, :]),
                                r32(flat(G[l][:, s, 4 * bank:4 * bank + 4, :])),
                                start=True, stop=False)
                        nc.tensor.matmul(
                            flat(ps[:, 8:12, :]), r32(ident[:, :]),
                            r32(flat(bnhbc[:, l, :, :])),
                            start=True, stop=False)
                        # W_hh @ h accumulation, bf16
                        for gt in range(GT):
                            for kt in range(KT):
                                nc.tensor.matmul(
                                    ps[:, gt, :],
                                    whhT[:, l, kt, gt * 128:(gt + 1) * 128],
                                    hb[:, kt, c0:c1],
                                    start=False, stop=(kt == KT - 1))
                        # gate math (fp32)
                        rz = work.tile([128, 8, HB], F32, tag=f"rz{s}")
                        nc.scalar.activation(rz[:, :, :], ps[:, 0:8, :],
                                             AF.Sigmoid)
                        tt = work.tile([128, 4, HB], F32, tag=f"t{s}")
                        nc.vector.tensor_tensor(tt[:, :, :], ps[:, 8:12, :],
                                                rz[:, 0:4, :], OP.mult)
                        uu = work.tile([128, 4, HB], F32, tag=f"u{s}")
                        nc.gpsimd.tensor_tensor(uu[:, :, :], tt[:, :, :],
                                                G[l][:, s, 8:12, :], OP.add)
                        nn_ = work.tile([128, 4, HB], F32, tag=f"n{s}")
                        nc.scalar.activation(nn_[:, :, :], uu[:, :, :], AF.Tanh)
                        aa = work.tile([128, 4, HB], F32, tag=f"a{s}")
                        nc.vector.tensor_tensor(aa[:, :, :], h32[:, :, c0:c1],
                                                nn_[:, :, :], OP.subtract)
                        bb = work.tile([128, 4, HB], F32, tag=f"b{s}")
                        nc.vector.tensor_tensor(bb[:, :, :], aa[:, :, :],
                                                rz[:, 4:8, :], OP.mult)
                        nc.vector.tensor_tensor(h32n[:, :, c0:c1], nn_[:, :, :],
                                                bb[:, :, :], OP.add)
                        nc.gpsimd.tensor_copy(hbn[:, :, c0:c1],
                                              h32n[:, :, c0:c1])
                    h32, hb = h32n, hbn

                # y_t = W_out @ h + b_out   -> [4, 256]
                py = pyp.tile([4, BL], F32, tag="y")
                for kt in range(KT):
                    nc.tensor.matmul(py[:, :], r32(woutT[:, kt, :]),
                                     r32(h32[:, kt, :]),
                                     start=(kt == 0), stop=False)
                nc.tensor.matmul(py[:, :], r32(bmi[32:33, 0:4]),
                                 r32(ones[32:33, :]), start=False, stop=True)
                if t % YCHUNK == 0:
                    ybuf = ybufp.tile([4, YCHUNK, BL], F32, tag="yb")
                nc.scalar.copy(ybuf[0:4, t % YCHUNK, :], py[:, :])
                if (t + 1) % YCHUNK == 0:
                    c0 = t + 1 - YCHUNK
                    nc.sync.dma_start(
                        y_d[c0:c0 + YCHUNK, :, :].rearrange("t o b -> o t b"),
                        ybuf[0:4, :, :])

    nc.compile()
    return nc


def _get_built():
    global _BUILT
    if _BUILT is None:
        _BUILT = _build()
    return _BUILT


def _pack(rows, n):
    out = np.zeros((128, n), np.float32)
    for i, r in enumerate(rows):
        out[32 * i, :] = np.asarray(r, np.float32)
    return out


def kernel(**inputs):
    global LAST_RESULTS, LAST_IN_MAPS
    nc = _get_built()

    f = np.float32
    bf = ml_dtypes.bfloat16
    dtp = np.ascontiguousarray(inputs["dtp_features"], dtype=f)
    shared = {
        "wencT": np.ascontiguousarray(
            inputs["W_enc"].T.reshape(ET, 128, H).astype(f)),
        "wihT": np.ascontiguousarray(np.stack(
            [inputs[f"W_ih{l}"].T.reshape(KT, 128, 3 * H) for l in "123"]).astype(f)),
        "whhT": np.ascontiguousarray(np.stack(
            [inputs[f"W_hh{l}"].T.reshape(KT, 128, 3 * H) for l in "123"]).astype(bf)),
        "woutT": np.ascontiguousarray(
            inputs["W_out"].T.reshape(KT, 128, 4).astype(f)),
        "brz": _pack([(inputs[f"b_ih{l}"] + inputs[f"b_hh{l}"])[:2 * H]
                      for l in "123"], 2 * H),
        "bni": _pack([inputs[f"b_ih{l}"][2 * H:] for l in "123"], H),
        "bmi": _pack([inputs["b_enc"], np.pad(inputs["b_out"], (0, H - 4))], H),
        "ident": np.eye(128, dtype=f),
        "ones": np.ones((128, BL), f),
        "bnhbc": np.ascontiguousarray(np.broadcast_to(
            np.stack([inputs[f"b_hh{l}"][2 * H:].reshape(4, 128).T
                      for l in "123"])[:, :, :, None],
            (3, 128, 4, HB)).astype(f)),
        "h0": np.zeros((128, KT, BL), f),
        "h0b": np.zeros((128, KT, BL), bf),
    }
    in_maps = []
    for c in range(NCORES):
        m = dict(shared)
        m["dtpT"] = np.ascontiguousarray(
            dtp[c * BL:(c + 1) * BL].T.reshape(ET, 128, BL))
        in_maps.append(m)

    LAST_IN_MAPS = in_maps
    res = run_bass_kernel_spmd(nc, in_maps, core_ids=list(range(NCORES)))
    LAST_RESULTS = res
    outs = []
    for c in range(NCORES):
        y = res.results[c]["y"]  # [60, 4, 256]
        outs.append(np.transpose(y, (2, 1, 0)).reshape(BL, 4 * STEPS))
    return np.ascontiguousarray(np.concatenate(outs, axis=0), dtype=np.float32)



# revision 3
# speedup vs baseline: 1.6431x; 1.6431x over previous
"""Trainium2 Bass kernel for the DecoderRNN-DTP problem.

Math (per reference):
  x   = relu(dtp_features @ W_enc.T + b_enc)              [B, H]
  gi_l = x @ W_ih_l.T + b_ih_l                            [B, 3H]   (constant over steps)
  60 steps, each step threads one hidden state h through 3 GRU cells:
      gh = h @ W_hh_l.T + b_hh_l
      r = sig(gi_r + gh_r); z = sig(gi_z + gh_z)
      n = tanh(gi_n + r * gh_n);  h = (1-z)*n + z*h
  y_t = h @ W_out.T + b_out                               [B, 4]
  out[b, o*60+t] = y_t[b, o]

Distribution: data-parallel over batch, 8 cores x 256 rows; weights replicated.

Layout is fully transposed on device: hidden/gate dims on SBUF partitions,
batch is the free/moving dim.  The 256-row per-core batch is split into two
128-row streams so the tensor engine runs one stream's W_hh matmuls while
the scalar/vector engines chew the other stream's gate math.

Everything flows in fp16 (PE runs fp16 at full rate; 16-bit dtypes unlock
the DVE 2x_1p mode so every tensor_tensor is ~2x faster; numpy study:
rel err ~8e-4 end-to-end).  PSUM accumulation is fp32.  Gate constants
(gi + biases) for r/z are preloaded into PSUM by bank-wide fp32r identity
matmuls so the W_hh accumulation lands on them; exactly one start=True per
PSUM bank (start clears has_written for the whole bank).

Critical-path trick: W_hh matmuls issue in r,n,z gate order, so sigmoid(r)
and tt = ps_n*r run on scalar/vector while the z-gate matmuls still occupy
the PE; after the block only uu -> tanh -> (1-z)*n -> +z*h remain exposed,
which fits inside the other stream's matmul block -> PE never stalls.
h' = (1-z)*n + z*h is computed as om = 1-z (tensor_scalar, 4x mode) and
q = z*h off the critical path, then p = om*n, h' = p + q.
"""

import numpy as np

import concourse.bass as bass
import concourse.bacc as bacc
import concourse.tile as tile
import concourse.mybir as mybir
from concourse.bass_utils import run_bass_kernel_spmd

H = 512
B = 2048
STEPS = 60
NCORES = 8
BL = B // NCORES   # 256 batch rows per core
HB = BL // 2       # 128 rows per stream
KT = H // 128      # 4 contraction tiles over hidden dim
GT = 3 * H // 128  # 12 gate tiles
ET = 2048 // 128   # 16 contraction tiles over encoder input dim
YCHUNK = 6         # steps buffered between output DMAs

F32 = mybir.dt.float32
F32R = mybir.dt.float32r
F16 = mybir.dt.float16
AF = mybir.ActivationFunctionType
OP = mybir.AluOpType

_BUILT = None
LAST_RESULTS = None
LAST_IN_MAPS = None

# W_hh matmul gate-tile issue order: r-gates, n-gates, z-gates (see module doc)
GT_ORDER = (0, 1, 2, 3, 8, 9, 10, 11, 4, 5, 6, 7)


def r32(ap):
    return ap.bitcast(F32R)


def flat(ap):
    return ap.rearrange("p a b -> p (a b)")


def _build():
    nc = bacc.Bacc("TRN2", target_bir_lowering=False, debug=False,
                   num_devices=NCORES)

    # ---- DRAM parameters (pre-laid-out on host) ----
    dtpT_d = nc.dram_tensor("dtpT", [ET, 128, BL], F16, kind="ExternalInput")
    wencT_d = nc.dram_tensor("wencT", [ET, 128, H], F16, kind="ExternalInput")
    wihT_d = nc.dram_tensor("wihT", [3, KT, 128, 3 * H], F16, kind="ExternalInput")
    whhT_d = nc.dram_tensor("whhT", [3, KT, 128, 3 * H], F16, kind="ExternalInput")
    woutT_d = nc.dram_tensor("woutT", [KT, 128, 4], F16, kind="ExternalInput")
    # biases packed with layer l at partition 32*l (K=1 matmul base_partition
    # must be 0/32/64); bmi: p0 = b_enc, p32 = b_out
    brz_d = nc.dram_tensor("brz", [128, 2 * H], F16, kind="ExternalInput")
    bni_d = nc.dram_tensor("bni", [128, H], F16, kind="ExternalInput")
    bmi_d = nc.dram_tensor("bmi", [128, H], F16, kind="ExternalInput")
    ident_d = nc.dram_tensor("ident", [128, 128], F32R, kind="ExternalInput")
    ones_d = nc.dram_tensor("ones", [128, BL], F16, kind="ExternalInput")
    # b_hh n-chunk broadcast across a 128-wide half-batch, per layer
    bnhbc_d = nc.dram_tensor("bnhbc", [3, 128, 4, HB], F32R, kind="ExternalInput")
    h0_d = nc.dram_tensor("h0", [128, KT, BL], F16, kind="ExternalInput")
    y_d = nc.dram_tensor("y", [STEPS, 4, BL], F32, kind="ExternalOutput")

    with tile.TileContext(nc) as tc:
        with (
            tc.tile_pool(name="consts", bufs=1) as consts,
            tc.tile_pool(name="whh", bufs=1) as whhp,
            tc.tile_pool(name="wstream", bufs=3) as wstream,
            tc.tile_pool(name="wihstream", bufs=2) as wihstream,
            tc.tile_pool(name="gpool", bufs=3) as gpool,
            tc.tile_pool(name="hpool", bufs=3) as hpool,
            tc.tile_pool(name="work", bufs=1) as work,
            tc.tile_pool(name="ybuf", bufs=2) as ybufp,
            tc.tile_pool(name="pg", bufs=1, space=bass.MemorySpace.PSUM) as pg,
            tc.tile_pool(name="py", bufs=2, space=bass.MemorySpace.PSUM) as pyp,
        ):
            # ---- constants ----
            ident = consts.tile([128, 128], F32R)
            nc.sync.dma_start(ident[:, :], ident_d[:, :])
            ones = consts.tile([128, BL], F16)
            nc.sync.dma_start(ones[:, :], ones_d[:, :])
            brz = consts.tile([128, 2 * H], F16)
            nc.sync.dma_start(brz[:, :], brz_d[:, :])
            bni = consts.tile([128, H], F16)
            nc.sync.dma_start(bni[:, :], bni_d[:, :])
            bmi = consts.tile([128, H], F16)
            nc.sync.dma_start(bmi[:, :], bmi_d[:, :])
            bnhbc = consts.tile([128, 3, 4, HB], F32R)
            for l in range(3):
                nc.sync.dma_start(bnhbc[:, l, :, :], bnhbc_d[l, :, :, :])
            woutT = consts.tile([128, KT, 4], F16)
            for kt in range(KT):
                nc.sync.dma_start(woutT[:, kt, :], woutT_d[kt, :, :])

            # ---- recurrent weights, fp16 (resident) ----
            whhT = whhp.tile([128, 3, KT, 3 * H], F16)
            for l in range(3):
                for kt in range(KT):
                    nc.sync.dma_start(whhT[:, l, kt, :], whhT_d[l, kt, :, :])

            # ---- encoder: xT = relu(W_enc @ dtp.T + b_enc), per stream ----
            # psx [128, KT, HB] f32 = 1 PSUM bank; start=True only on the
            # first matmul touching the bank (clears has_written bank-wide).
            psx = [pg.tile([128, KT, HB], F32, tag=f"g{s}", name=f"psx{s}")
                   for s in (0, 1)]
            for et in range(ET):
                dtc = wstream.tile([128, BL], F16, tag="dtp")
                nc.sync.dma_start(dtc[:, :], dtpT_d[et, :, :])
                wec = wstream.tile([128, H], F16, tag="wenc")
                nc.sync.dma_start(wec[:, :], wencT_d[et, :, :])
                for s in (0, 1):
                    for ht in range(KT):
                        nc.tensor.matmul(psx[s][:, ht, :],
                                         wec[:, ht * 128:(ht + 1) * 128],
                                         dtc[:, s * HB:(s + 1) * HB],
                                         start=(et == 0 and ht == 0),
                                         stop=False)
            for s in (0, 1):
                for ht in range(KT):
                    nc.tensor.matmul(psx[s][:, ht, :],
                                     bmi[0:1, ht * 128:(ht + 1) * 128],
                                     ones[0:1, 0:HB],
                                     start=False, stop=True)
            xT = work.tile([128, KT, BL], F16, tag="xT")
            for s in (0, 1):
                nc.scalar.activation(xT[:, :, s * HB:(s + 1) * HB],
                                     psx[s][:, :, :], AF.Relu)

            # ---- G_l: rz part fp32 (PSUM preload source), n part fp16 ----
            G = []
            Gn = []
            for l in range(3):
                psg = [pg.tile([128, GT, HB], F32, tag=f"g{s}",
                               name=f"psg{l}_{s}") for s in (0, 1)]
                for kt in range(KT):
                    wic = wihstream.tile([128, 3 * H], F16, tag="wih")
                    nc.sync.dma_start(wic[:, :], wihT_d[l, kt, :, :])
                    for s in (0, 1):
                        for gt in range(GT):
                            nc.tensor.matmul(
                                psg[s][:, gt, :],
                                wic[:, gt * 128:(gt + 1) * 128],
                                xT[:, kt, s * HB:(s + 1) * HB],
                                start=(kt == 0 and gt % 4 == 0),
                                stop=False)
                for s in (0, 1):
                    for gt in range(8):
                        nc.tensor.matmul(psg[s][:, gt, :],
                                         brz[32 * l:32 * l + 1,
                                             gt * 128:(gt + 1) * 128],
                                         ones[32 * l:32 * l + 1, 0:HB],
                                         start=False, stop=True)
                    for j in range(4):
                        nc.tensor.matmul(psg[s][:, 8 + j, :],
                                         bni[32 * l:32 * l + 1,
                                             j * 128:(j + 1) * 128],
                                         ones[32 * l:32 * l + 1, 0:HB],
                                         start=False, stop=True)
                g = gpool.tile([128, 2, 8, HB], F32R, tag="G", name=f"G{l}")
                gn = gpool.tile([128, 2, 4, HB], F16, tag="Gn", name=f"Gn{l}")
                for s in (0, 1):
                    nc.scalar.copy(g[:, s, :, :], psg[s][:, 0:8, :])
                    nc.scalar.copy(gn[:, s, :, :], psg[s][:, 8:12, :])
                G.append(g)
                Gn.append(gn)

            # ---- recurrent loop: 60 steps x 3 GRU cells, 2 streams ----
            hb = hpool.tile([128, KT, BL], F16, tag="h")
            nc.sync.dma_start(hb[:, :, :], h0_d[:, :, :])

            hb_y = None     # layer-3 h of the previous step (y-head input)
            py = None
            ybuf = None

            def emit_y_half(t, s):
                nonlocal py, ybuf
                c0, c1 = s * HB, (s + 1) * HB
                if s == 0:
                    py = pyp.tile([4, BL], F32, tag="y")
                for kt in range(KT):
                    nc.tensor.matmul(py[:, c0:c1], woutT[:, kt, :],
                                     hb_y[:, kt, c0:c1],
                                     start=(s == 0 and kt == 0), stop=False)
                nc.tensor.matmul(py[:, c0:c1], bmi[32:33, 0:4],
                                 ones[32:33, c0:c1], start=False, stop=True)
                if s == 1:
                    if t % YCHUNK == 0:
                        ybuf = ybufp.tile([4, YCHUNK, BL], F32, tag="yb")
                    nc.scalar.copy(ybuf[0:4, t % YCHUNK, :], py[:, :])
                    if (t + 1) % YCHUNK == 0:
                        c = t + 1 - YCHUNK
                        nc.sync.dma_start(
                            y_d[c:c + YCHUNK, :, :].rearrange("t o b -> o t b"),
                            ybuf[0:4, :, :])

            for t in range(STEPS):
                for l in range(3):
                    hbn = hpool.tile([128, KT, BL], F16, tag="h")
                    for s in (0, 1):
                        c0, c1 = s * HB, (s + 1) * HB
                        # y-head for the previous step, slotted here so the
                        # producing gate chains are long done (no PE stall)
                        if l == 0 and t > 0:
                            emit_y_half(t - 1, s)
                        ps = pg.tile([128, GT, HB], F32, tag=f"g{s}")
                        # bank-wide gate-constant preloads (one start=True
                        # per 2KB PSUM bank)
                        nc.tensor.matmul(
                            flat(ps[:, 0:4, :]), ident[:, :],
                            r32(flat(G[l][:, s, 0:4, :])),
                            start=True, stop=False)
                        nc.tensor.matmul(
                            flat(ps[:, 8:12, :]), ident[:, :],
                            flat(bnhbc[:, l, :, :]),
                            start=True, stop=False)
                        nc.tensor.matmul(
                            flat(ps[:, 4:8, :]), ident[:, :],
                            r32(flat(G[l][:, s, 4:8, :])),
                            start=True, stop=False)
                        # W_hh @ h accumulation, fp16, r/n/z gate order
                        for gt in GT_ORDER:
                            for kt in range(KT):
                                nc.tensor.matmul(
                                    ps[:, gt, :],
                                    whhT[:, l, kt, gt * 128:(gt + 1) * 128],
                                    hb[:, kt, c0:c1],
                                    start=False, stop=(kt == KT - 1))
                        # gate math, fp16 (DVE 2x)
                        r16 = work.tile([128, 4, HB], F16, tag=f"r{s}")
                        nc.scalar.activation(r16[:, :, :], ps[:, 0:4, :],
                                             AF.Sigmoid)
                        tt = work.tile([128, 4, HB], F16, tag=f"t{s}")
                        nc.vector.tensor_tensor(tt[:, :, :], ps[:, 8:12, :],
                                                r16[:, :, :], OP.mult)
                        z16 = work.tile([128, 4, HB], F16, tag=f"z{s}")
                        nc.scalar.activation(z16[:, :, :], ps[:, 4:8, :],
                                             AF.Sigmoid)
                        uu = work.tile([128, 4, HB], F16, tag=f"u{s}")
                        nc.vector.tensor_tensor(uu[:, :, :], tt[:, :, :],
                                                Gn[l][:, s, :, :], OP.add)
                        nn_ = work.tile([128, 4, HB], F16, tag=f"n{s}")
                        nc.scalar.activation(nn_[:, :, :], uu[:, :, :], AF.Tanh)
                        qq = work.tile([128, 4, HB], F16, tag=f"q{s}")
                        nc.vector.tensor_tensor(qq[:, :, :], hb[:, :, c0:c1],
                                                z16[:, :, :], OP.mult)
                        om = work.tile([128, 4, HB], F16, tag=f"o{s}")
                        nc.vector.tensor_scalar(om[:, :, :], z16[:, :, :],
                                                -1.0, 1.0, OP.mult, OP.add)
                        pp = work.tile([128, 4, HB], F16, tag=f"p{s}")
                        nc.vector.tensor_tensor(pp[:, :, :], om[:, :, :],
                                                nn_[:, :, :], OP.mult)
                        nc.vector.tensor_tensor(hbn[:, :, c0:c1], pp[:, :, :],
                                                qq[:, :, :], OP.add)
                    hb = hbn
                hb_y = hb
            for s in (0, 1):
                emit_y_half(STEPS - 1, s)

    nc.compile()
    return nc


def _get_built():
    global _BUILT
    if _BUILT is None:
        _BUILT = _build()
    return _BUILT


def _pack(rows, n):
    out = np.zeros((128, n), np.float16)
    for i, r in enumerate(rows):
        out[32 * i, :] = np.asarray(r, np.float16)
    return out


def kernel(**inputs):
    global LAST_RESULTS, LAST_IN_MAPS
    nc = _get_built()

    f = np.float32
    f16 = np.float16
    dtp = np.ascontiguousarray(inputs["dtp_features"], dtype=f)
    shared = {
        "wencT": np.ascontiguousarray(
            inputs["W_enc"].T.reshape(ET, 128, H).astype(f16)),
        "wihT": np.ascontiguousarray(np.stack(
            [inputs[f"W_ih{l}"].T.reshape(KT, 128, 3 * H) for l in "123"]).astype(f16)),
        "whhT": np.ascontiguousarray(np.stack(
            [inputs[f"W_hh{l}"].T.reshape(KT, 128, 3 * H) for l in "123"]).astype(f16)),
        "woutT": np.ascontiguousarray(
            inputs["W_out"].T.reshape(KT, 128, 4).astype(f16)),
        "brz": _pack([(inputs[f"b_ih{l}"] + inputs[f"b_hh{l}"])[:2 * H]
                      for l in "123"], 2 * H),
        "bni": _pack([inputs[f"b_ih{l}"][2 * H:] for l in "123"], H),
        "bmi": _pack([inputs["b_enc"], np.pad(inputs["b_out"], (0, H - 4))], H),
        "ident": np.eye(128, dtype=f),
        "ones": np.ones((128, BL), f16),
        "bnhbc": np.ascontiguousarray(np.broadcast_to(
            np.stack([inputs[f"b_hh{l}"][2 * H:].reshape(4, 128).T
                      for l in "123"])[:, :, :, None],
            (3, 128, 4, HB)).astype(f)),
        "h0": np.zeros((128, KT, BL), f16),
    }
    in_maps = []
    for c in range(NCORES):
        m = dict(shared)
        m["dtpT"] = np.ascontiguousarray(
            dtp[c * BL:(c + 1) * BL].T.reshape(ET, 128, BL).astype(f16))
        in_maps.append(m)

    LAST_IN_MAPS = in_maps
    res = run_bass_kernel_spmd(nc, in_maps, core_ids=list(range(NCORES)))
    LAST_RESULTS = res
    outs = []
    for c in range(NCORES):
        y = res.results[c]["y"]  # [60, 4, 256]
        outs.append(np.transpose(y, (2, 1, 0)).reshape(BL, 4 * STEPS))
    return np.ascontiguousarray(np.concatenate(outs, axis=0), dtype=np.float32)
